# revision 6
# baseline (speedup 1.0000x reference)
"""Trainium2 Bass kernel for ContrastiveMaskedPatchSimilarity loss.

Computes: per-position cosine similarity along the channel axis of two
[32, 256, 64, 64] f32 tensors, then a masked mean -> scalar.

Strategy (pure data parallel over 8 NeuronCores, batch-sharded 4 each):
  - Inputs are cast to fp16 on the host before upload: HBM traffic halves
    (the kernel is DMA-bound) and fp16's 11-bit mantissa keeps the final
    rel-err ~1.6e-4 (better than bf16 products from f32, which give 6.7e-4).
  - Layout on chip: [channel-chunk (128) = partitions, spatial (4096) = free].
    DMA tiles are contiguous 8KB-per-partition lines.
  - Elementwise products (u*m, u*u, m*m) -> fp16 on DVE (2x 16-bit mode)
    and ACT, balanced so neither exceeds the DMA roofline.
  - Channel reduction via TensorE: product slice [128ch x 128pos] is the
    stationary operand, rhs = ones[128,1] fp16 -> out[128pos, 1] in PSUM.
    The two channel chunks accumulate into the same PSUM column
    (start/stop flags), so the epilogue needs no cross-chunk adds.
  - Epilogue per batch (delayed one batch so ACT never stalls on PE):
    num/(sqrt(uu*mm)), then a fused multiply-by-mask + free-axis reduce
    (tensor_tensor_reduce) -> acc[:, b].
  - Host: sum partials over cores/partitions, divide by host-computed
    mask count (exact).
"""

import sys
from contextlib import ExitStack

import numpy as np

sys.path.insert(0, "/opt/trn_rl_repo")

import concourse.bass as bass  # noqa: E402
import concourse.tile as tile  # noqa: E402
from concourse import bacc, mybir  # noqa: E402
from concourse.bass_utils import run_bass_kernel_spmd  # noqa: E402

B, C, H, W = 32, 256, 64, 64
NCORES = 8
BL = B // NCORES  # batches per core: 4
HWX = H * W  # 4096
ROWS = BL * C  # 1024
NPB = HWX // 128  # position blocks per batch: 32
NCHUNK = C // 128  # channel chunks: 2

F32 = mybir.dt.float32
F16 = mybir.dt.float16

_CACHED_NC = None


def build_nc():
    nc = bacc.Bacc(
        "TRN2", target_bir_lowering=False, debug=False, num_devices=NCORES
    )
    u_d = nc.dram_tensor("u", [ROWS, HWX], F16, kind="ExternalInput")
    m_d = nc.dram_tensor("m", [ROWS, HWX], F16, kind="ExternalInput")
    # mask, pre-transposed on host to [p_in (128), b*NPB + pb (128)] f32
    mk_d = nc.dram_tensor("maskf", [128, BL * NPB], F32, kind="ExternalInput")
    ones_d = nc.dram_tensor("ones", [128, 1], F16, kind="ExternalInput")
    # out[:, b] = per-batch sum(sim*mask) partials (per partition)
    out_d = nc.dram_tensor("out", [128, BL], F32, kind="ExternalOutput")

    with tile.TileContext(nc) as tc, ExitStack() as ctx:
        const_pool = ctx.enter_context(tc.tile_pool(name="const", bufs=1))
        in_pool = ctx.enter_context(tc.tile_pool(name="inp", bufs=6))
        prod_pool = ctx.enter_context(tc.tile_pool(name="prod", bufs=2))
        ep_pool = ctx.enter_context(tc.tile_pool(name="ep", bufs=2))
        acc_pool = ctx.enter_context(tc.tile_pool(name="acc", bufs=1))
        psum_pool = ctx.enter_context(
            tc.tile_pool(name="psum", bufs=2, space="PSUM")
        )

        ones_t = const_pool.tile([128, 1], F16)
        nc.sync.dma_start(ones_t[:], ones_d[:, :])
        maskf_t = const_pool.tile([128, BL * NPB], F32)
        nc.sync.dma_start(maskf_t[:], mk_d[:, :])
        acc_t = acc_pool.tile([128, BL], F32)

        def epilogue(b, P):
            # PSUM cols of P: num [0:NPB], uu [NPB:2NPB], mm [2NPB:3NPB]
            # DVE has a single PSUM read port: ACT copies num/uu to SBUF,
            # then each DVE op touches at most one PSUM operand.
            nn = ep_pool.tile([128, NPB], F32, tag="nn")
            nc.scalar.copy(nn[:], P[:, 0:NPB])
            us = ep_pool.tile([128, NPB], F32, tag="us")
            nc.scalar.copy(us[:], P[:, NPB : 2 * NPB])
            d2 = ep_pool.tile([128, NPB], F32, tag="d2")
            nc.vector.tensor_mul(d2[:], us[:], P[:, 2 * NPB : 3 * NPB])
            r = ep_pool.tile([128, NPB], F32, tag="r")
            nc.vector.reciprocal(r[:], d2[:])
            rs = ep_pool.tile([128, NPB], F32, tag="rs")
            nc.scalar.sqrt(rs[:], r[:])
            simv = ep_pool.tile([128, NPB], F32, tag="simv")
            nc.vector.tensor_mul(simv[:], nn[:], rs[:])
            # (tensor_tensor_reduce would fuse these two, but it faults on
            # this HW stack — keep the plain pair)
            sm = ep_pool.tile([128, NPB], F32, tag="sm")
            nc.vector.tensor_mul(
                sm[:], simv[:], maskf_t[:, b * NPB : (b + 1) * NPB]
            )
            nc.vector.tensor_reduce(
                acc_t[:, b : b + 1],
                sm[:],
                axis=mybir.AxisListType.X,
                op=mybir.AluOpType.add,
            )

        pend = []  # (b, P) awaiting epilogue
        for b in range(BL):
            # PSUM cols: s*NPB + pb, both channel chunks accumulate in place
            P = psum_pool.tile([128, 3 * NPB], F32)
            prods = []  # [ch][stat] product tiles
            for ch in range(NCHUNK):
                row0 = b * C + ch * 128
                u_t = in_pool.tile([128, HWX], F16, tag="u")
                nc.sync.dma_start(u_t[:], u_d[row0 : row0 + 128, :])
                m_t = in_pool.tile([128, HWX], F16, tag="m")
                nc.gpsimd.dma_start(m_t[:], m_d[row0 : row0 + 128, :])

                num_t = prod_pool.tile([128, HWX], F16, tag="num")
                nc.vector.tensor_mul(num_t[:], u_t[:], m_t[:])
                uu_t = prod_pool.tile([128, HWX], F16, tag="uu")
                nc.scalar.square(uu_t[:], u_t[:])
                mm_t = prod_pool.tile([128, HWX], F16, tag="mm")
                # m*m stays on DVE: fp16 2x mode makes DVE ~1.6x faster per
                # product than ACT (which runs 1 col/cycle at any dtype), so
                # DVE num+mm (~37us) vs ACT uu+epilogue (~34us) balances
                nc.vector.tensor_mul(mm_t[:], m_t[:], m_t[:])
                prods.append((num_t, uu_t, mm_t))

                # epilogue of the previous batch after chunk-0 products, so
                # ACT keeps streaming while PE finishes batch b-1
                if ch == 0 and pend:
                    epilogue(*pend.pop())

            # both chunks ready: per PSUM column, adjacent accumulation pair
            for s in range(3):
                for pb in range(NPB):
                    col = s * NPB + pb
                    csl = slice(pb * 128, (pb + 1) * 128)
                    nc.tensor.matmul(
                        P[:, col : col + 1],
                        prods[0][s][:, csl],
                        ones_t[:, :],
                        start=True,
                        stop=False,
                    )
                    nc.tensor.matmul(
                        P[:, col : col + 1],
                        prods[1][s][:, csl],
                        ones_t[:, :],
                        start=False,
                        stop=True,
                    )
            pend.append((b, P))

        epilogue(*pend.pop())
        nc.sync.dma_start(out_d[:, :], acc_t[:])

    nc.compile()
    return nc


def get_nc():
    global _CACHED_NC
    if _CACHED_NC is None:
        _CACHED_NC = build_nc()
    return _CACHED_NC


def make_in_maps(unmasked, masked, latent_mask):
    ones = np.ones((128, 1), dtype=np.float16)
    u16 = unmasked.astype(np.float16)
    m16 = masked.astype(np.float16)
    in_maps = []
    for i in range(NCORES):
        sl = slice(i * BL, (i + 1) * BL)
        u = np.ascontiguousarray(u16[sl]).reshape(ROWS, HWX)
        m = np.ascontiguousarray(m16[sl]).reshape(ROWS, HWX)
        mk = latent_mask[sl].reshape(128, 128).T.astype(np.float32)
        in_maps.append(
            {
                "u": u,
                "m": m,
                "maskf": np.ascontiguousarray(mk),
                "ones": ones,
            }
        )
    return in_maps


def _finalize(results, latent_mask):
    num = 0.0
    for res in results:
        num += np.asarray(res["out"], dtype=np.float64).sum()
    den = float((latent_mask != 0).sum())
    return np.float32(num / den)


def kernel(unmasked_latent_tensors, masked_latent_tensors, latent_mask, **kw):
    nc = get_nc()
    lm = np.asarray(latent_mask)
    in_maps = make_in_maps(
        np.asarray(unmasked_latent_tensors, dtype=np.float32),
        np.asarray(masked_latent_tensors, dtype=np.float32),
        lm,
    )
    res = run_bass_kernel_spmd(nc, in_maps, list(range(NCORES)))
    return _finalize(res.results, lm)


def kernel_traced(unmasked_latent_tensors, masked_latent_tensors, latent_mask):
    """Like kernel() but with NTFF tracing; returns (value, BassKernelResults)."""
    nc = get_nc()
    lm = np.asarray(latent_mask)
    in_maps = make_in_maps(
        np.asarray(unmasked_latent_tensors, dtype=np.float32),
        np.asarray(masked_latent_tensors, dtype=np.float32),
        lm,
    )
    res = run_bass_kernel_spmd(nc, in_maps, list(range(NCORES)), trace=True)
    return _finalize(res.results, lm), res
